# revision 8
# baseline (speedup 1.0000x reference)
"""Gemma4 MoE feed-forward on 8 Trainium2 NeuronCores.

Strategy: expert-parallel. E == n_cores == 8, so core e owns expert e's
weights (Wg[e], Wu[e], Wd[e]) and receives exactly the tokens routed to
expert e (gathered + transposed + padded on the host). Each core runs a
dense gated-FFN over its token batch:

    dT = Wd^T @ (gelu_tanh(Wg^T x^T) * (Wu^T x^T))        (all [*, C] layouts)

The host then scatter-adds routing_weight * dT^T back into the full
[T, H] output. Tokens that select the same expert in both slots are
deduplicated on the host (weights summed).

All matmul operands are bf16 (rel err ~4e-3 vs the 2e-2 gate): bf16
streams the moving operand ~20ns/MM faster than fp32r and halves HBM
traffic. The up phase runs i-outer / n-inner so each Wg/Wu tile is
DMA'd exactly once (n-outer re-streamed all up weights per token block
and saturated the 358 GB/s per-core HBM budget). DMAs are consolidated
into few large transfers: each dma_start costs ~650ns of serial
sequencer issue time, so the startup ramp is paced by issue count as
much as by bytes.
"""

import os
import sys

import numpy as np

for _p in ("/opt/trn_rl_repo", "/root/.axon_site/_ro/trn_rl_repo"):
    if os.path.isdir(_p) and _p not in sys.path:
        sys.path.append(_p)

T, H, I, E, K = 4096, 2048, 1024, 8, 2
NCORES = 8
KH = H // 128  # 16 k-tiles over the hidden dim
KI = I // 128  # 8 k-tiles over the intermediate dim

_PROGRAM_CACHE = {}
LAST_RESULT = None  # BassKernelResults of the most recent run (for test.py)
TRACE = False  # test.py sets this to capture an NTFF profile
TRACE_CORES = [0]

NWARM = int(os.environ.get("MOE_NWARM", "7"))
OTAIL = 128  # final down-group split width (shortens the last write chain)


def _tile_w_up(W):
    """[H, I] -> [KI, 128, KH*128] with [i, p, k*128+c] = W[k*128+p, i*128+c]:
    the i-tile's whole lhsT set is one DMA, 4KB contiguous per partition."""
    return np.ascontiguousarray(
        W.reshape(KH, 128, KI, 128).transpose(2, 1, 0, 3).reshape(KI, 128, KH * 128)
    )


def _tile_w_down(W):
    """[I, H] -> [KH, 128, KI*128], same scheme (contraction over I)."""
    return np.ascontiguousarray(
        W.reshape(KI, 128, KH, 128).transpose(2, 1, 0, 3).reshape(KH, 128, KI * 128)
    )


def _pick_config(max_count):
    """Minimal uniform token-block config: NT blocks of even width N with
    NT*N >= max_count, N <= 512 (PSUM bank limit) and N >= 256 (so the
    ~100ns LDWEIGHTS fully hides under the moving-operand stream)."""
    mc = max(max_count, 256)
    nt = -(-mc // 512)
    n = -(-mc // nt)
    n += n % 2
    return (nt * n, nt, n)  # (C, NT, N)


def _build_program(C, NT, N):
    import concourse.tile as tile
    from concourse import bacc, mybir
    from contextlib import ExitStack

    f32 = mybir.dt.float32
    bf16 = mybir.dt.bfloat16

    nc = bacc.Bacc("TRN2", target_bir_lowering=False, debug=False)

    # x arrives host-packed: [n, p, k*N+c] = x^T[k*128+p, n*N+c], so each
    # n-block is one contiguous KH*N*2 = ~16KB run per partition.
    xP = nc.dram_tensor("xP", [NT, 128, KH * N], bf16, kind="ExternalInput").ap()
    Wg_d = nc.dram_tensor("Wg", [KI, 128, KH * 128], bf16, kind="ExternalInput").ap()
    Wu_d = nc.dram_tensor("Wu", [KI, 128, KH * 128], bf16, kind="ExternalInput").ap()
    Wd_d = nc.dram_tensor("Wd", [KH, 128, KI * 128], bf16, kind="ExternalInput").ap()
    dT = nc.dram_tensor("dT", [H, C], f32, kind="ExternalOutput").ap()

    # Partition-major view: row a*128+p -> partition p, free index a.
    dT_p = dT.rearrange("(a p) c -> p a c", p=128)  # [128, KH, C]

    GELU = mybir.ActivationFunctionType.Gelu_apprx_tanh

    with tile.TileContext(nc) as tc, ExitStack() as ctx:
        xpool = ctx.enter_context(tc.tile_pool(name="x", bufs=1))
        wpool = ctx.enter_context(tc.tile_pool(name="w", bufs=3))
        apool = ctx.enter_context(tc.tile_pool(name="a", bufs=1))
        tpool = ctx.enter_context(tc.tile_pool(name="t", bufs=2))
        opool = ctx.enter_context(tc.tile_pool(name="o", bufs=4))
        wdpool = ctx.enter_context(tc.tile_pool(name="wd", bufs=4))

        # PE clock-gate warmup: HAM starts at 1.2 GHz and un-throttles only
        # after ~3.4us of sustained activity. Real matmuls can't start until
        # the first weights + x block land from HBM (~16us: ~8us framework
        # preamble before the first DMA byte moves, then ~3MB at the HBM
        # rate). Dummy bf16 matmuls on memset scratch need no DMA, so they
        # run right at launch and the real stream begins at 2.4 GHz. The
        # count is sized to end just as the first real matmul's operands
        # land: shorter leaves the real stream DMA-paced with PE gaps (HAM
        # re-throttles, costing ~5us); longer delays the stream (PE queue
        # is FIFO).
        with (
            tc.tile_pool(name="warm", bufs=1) as wmpool,
            tc.tile_pool(name="warmps", bufs=1, space="PSUM") as wmpspool,
        ):
            wt = wmpool.tile([128, 512], bf16, name="warm_in")
            nc.vector.memset(wt[:], 0.0)
            wps = wmpspool.tile([128, 512], f32, name="warm_ps")
            for r in range(NWARM):
                nc.tensor.matmul(wps[:], wt[:, 0:128], wt[:], start=True, stop=True)

        w_tiles = {}

        def issue_w(i):
            wgt = wpool.tile([128, KH * 128], bf16, tag="wg", name=f"wg{i}")
            wut = wpool.tile([128, KH * 128], bf16, tag="wu", name=f"wu{i}")
            nc.sync.dma_start(wgt[:], Wg_d[i])
            nc.sync.dma_start(wut[:], Wu_d[i])
            w_tiles[i] = (wgt, wut)

        xts = {}

        def alloc_x(n):
            t = xpool.tile([128, KH * N], bf16, name=f"x{n}")
            xts[n] = t
            return t

        def issue_x_part(n, k0, k1):
            nc.sync.dma_start(
                xts[n][:, k0 * N : k1 * N], xP[n, :, k0 * N : k1 * N]
            )

        # Ramp emission order (sync ring is FIFO, drained in issue order):
        # the i=0 weights and n=0 x block are split into halves/quarters
        # interleaved in the order group (0,0) consumes them — the tile
        # framework tracks dependencies at slice granularity, so the real
        # stream starts as soon as the first ~1MB lands (~12.5us) instead
        # of waiting for the whole first-group working set. Then the
        # remaining x blocks, then the i=1,2 weight prefetches (x must
        # outrank them or the stream stalls at (i=0, n=1)).
        wg0 = wpool.tile([128, KH * 128], bf16, tag="wg", name="wg0")
        wu0 = wpool.tile([128, KH * 128], bf16, tag="wu", name="wu0")
        w_tiles[0] = (wg0, wu0)
        hw = (KH // 2) * 128
        for n in range(NT):
            alloc_x(n)
        nc.sync.dma_start(wg0[:, 0:hw], Wg_d[0, :, 0:hw])
        nc.sync.dma_start(wu0[:, 0:hw], Wu_d[0, :, 0:hw])
        issue_x_part(0, 0, KH // 4)
        issue_x_part(0, KH // 4, KH // 2)
        nc.sync.dma_start(wg0[:, hw:], Wg_d[0, :, hw:])
        nc.sync.dma_start(wu0[:, hw:], Wu_d[0, :, hw:])
        issue_x_part(0, KH // 2, 3 * KH // 4)
        issue_x_part(0, 3 * KH // 4, KH)
        for n in range(1, NT):
            for q in range(4):
                issue_x_part(n, q * KH // 4, (q + 1) * KH // 4)
        for i in (1, 2):
            if i < KI:
                issue_w(i)

        aT = apool.tile([128, KI, C], bf16, name="aT")

        wd_tiles = {}

        def issue_wd(h):
            wdt = wdpool.tile([128, KI * 128], bf16, tag="wd", name=f"wd{h}")
            nc.sync.dma_start(wdt[:], Wd_d[h])
            wd_tiles[h] = wdt

        # Both PSUM pools stay open for the whole kernel (2*2 + 4 = 8
        # banks): closing gu before opening d would insert a drain barrier
        # (~1.5-3.5us of PE idle at the up->down transition).
        with (
            tc.tile_pool(name="gu", bufs=2, space="PSUM") as gupool,
            tc.tile_pool(name="d", bufs=4, space="PSUM") as dpool,
        ):
            # Up phase: i-outer so each weight tile is loaded exactly once.
            for i in range(KI):
                if i + 3 < KI and i + 3 not in w_tiles:
                    issue_w(i + 3)
                # prefetch the first down-phase weights near the end
                if i >= KI - 3 and (h := i - (KI - 3)) < 3:
                    issue_wd(h)
                if i not in w_tiles:
                    issue_w(i)
                wgt, wut = w_tiles.pop(i)
                for n in range(NT):
                    g_ps = gupool.tile([128, N], f32, tag="g", name=f"g{i}_{n}")
                    u_ps = gupool.tile([128, N], f32, tag="u", name=f"u{i}_{n}")
                    nsl = slice(n * N, (n + 1) * N)
                    for k in range(KH):
                        ksl = slice(k * 128, (k + 1) * 128)
                        xs = xts[n][:, k * N : (k + 1) * N]
                        nc.tensor.matmul(
                            g_ps[:], wgt[:, ksl], xs,
                            start=(k == 0), stop=(k == KH - 1),
                        )
                        nc.tensor.matmul(
                            u_ps[:], wut[:, ksl], xs,
                            start=(k == 0), stop=(k == KH - 1),
                        )
                    gel = tpool.tile([128, N], f32, tag="gel", name=f"gel{i}_{n}")
                    nc.scalar.activation(gel[:], g_ps[:], GELU)
                    nc.vector.tensor_mul(aT[:, i, nsl], gel[:], u_ps[:])

            # Down phase: d^T[h] = sum_ki Wd[ki,h]^T @ aT[ki]. Output DMAs
            # alternate between the two HWDGE rings (nc.scalar / nc.sync)
            # so HBM write receipts overlap and neither ring backs up. The
            # very last group is split into a wide piece and an OTAIL-wide
            # piece so the final copy+DMA+receipt chain is short.
            for h in range(KH):
                if h + 3 < KH and h + 3 not in wd_tiles:
                    issue_wd(h + 3)
                if h not in wd_tiles:
                    issue_wd(h)
                wdt = wd_tiles.pop(h)
                for n in range(NT):
                    last = h == KH - 1 and n == NT - 1
                    splits = (
                        [(0, N - OTAIL), (N - OTAIL, N)] if last else [(0, N)]
                    )
                    for si, (c0, c1) in enumerate(splits):
                        d_ps = dpool.tile(
                            [128, c1 - c0], f32, tag="d", name=f"d{h}_{n}_{si}"
                        )
                        for ki in range(KI):
                            nc.tensor.matmul(
                                d_ps[:],
                                wdt[:, ki * 128 : (ki + 1) * 128],
                                aT[:, ki, n * N + c0 : n * N + c1],
                                start=(ki == 0),
                                stop=(ki == KI - 1),
                            )
                        o = opool.tile(
                            [128, c1 - c0], f32, tag="o", name=f"o{h}_{n}_{si}"
                        )
                        nc.vector.tensor_copy(o[:], d_ps[:])
                        eng = nc.scalar if (h * NT + n + si) % 2 == 0 else nc.sync
                        eng.dma_start(
                            dT_p[:, h, n * N + c0 : n * N + c1], o[:]
                        )

    nc.compile()
    return nc


def _get_program(C, NT, N):
    key = (C, NT, N, NWARM)
    if key not in _PROGRAM_CACHE:
        _PROGRAM_CACHE[key] = _build_program(C, NT, N)
    return _PROGRAM_CACHE[key]


def _ensure_ntff_hook():
    """Register the axon NTFF profile hook if the image's antenv lacks
    axon_hooks (see trn_agent_boot.trn_boot). Only needed when TRACE."""
    import types

    try:
        from antenv.axon_hooks import get_axon_ntff_profile_hook  # noqa: F401

        return
    except ImportError:
        pass
    import antenv
    from trn_agent_boot.trn_boot import _ntff_profile_via_ctypes

    hook = _ntff_profile_via_ctypes("/opt/axon/libaxon_pjrt.so")
    mod = types.ModuleType("antenv.axon_hooks")
    state = {"hook": hook}
    mod.set_axon_ntff_profile_hook = lambda h: state.__setitem__("hook", h)
    mod.get_axon_ntff_profile_hook = lambda: state["hook"]
    sys.modules["antenv.axon_hooks"] = mod
    antenv.axon_hooks = mod


def kernel(x, Wg, Wu, Wd, selected_experts, routing_weights):
    global LAST_RESULT
    import ml_dtypes
    from concourse.bass_utils import run_bass_kernel_spmd

    if TRACE:
        _ensure_ntff_hook()

    bf16 = ml_dtypes.bfloat16
    x = np.asarray(x, dtype=np.float32)
    Wg = np.asarray(Wg, dtype=np.float32)
    Wu = np.asarray(Wu, dtype=np.float32)
    Wd = np.asarray(Wd, dtype=np.float32)
    selected_experts = np.asarray(selected_experts)
    routing_weights = np.asarray(routing_weights, dtype=np.float32)

    # Host-side dispatch: per expert, the (deduplicated) token list and
    # summed routing weights.
    idx_list, w_list = [], []
    for e in range(E):
        m = selected_experts == e  # [T, K]
        idx = np.nonzero(m.any(axis=1))[0]
        w = (routing_weights * m).sum(axis=1)[idx]
        idx_list.append(idx)
        w_list.append(w.astype(np.float32))

    max_count = max(len(idx) for idx in idx_list)
    C, NT, N = _pick_config(max_count)

    nc = _get_program(C, NT, N)

    in_maps = []
    for e in range(E):
        idx = idx_list[e]
        xT = np.zeros((H, C), dtype=bf16)
        xT[:, : len(idx)] = x[idx].T.astype(bf16)
        # pack [n, p, k*N+c] = xT[k*128+p, n*N+c]
        xPk = (
            xT.reshape(KH, 128, NT, N).transpose(2, 1, 0, 3).reshape(NT, 128, KH * N)
        )
        in_maps.append(
            {
                "xP": np.ascontiguousarray(xPk),
                "Wg": _tile_w_up(Wg[e].astype(bf16)),
                "Wu": _tile_w_up(Wu[e].astype(bf16)),
                "Wd": _tile_w_down(Wd[e].astype(bf16)),
            }
        )

    res = run_bass_kernel_spmd(
        nc,
        in_maps,
        list(range(NCORES)),
        trace=TRACE,
        trace_cores=TRACE_CORES if TRACE else None,
    )
    LAST_RESULT = res

    out = np.zeros((T, H), dtype=np.float32)
    for e in range(E):
        idx = idx_list[e]
        dTe = res.results[e]["dT"]  # [H, C] fp32
        out[idx] += w_list[e][:, None] * dTe[:, : len(idx)].T
    return out


# revision 12
# speedup vs baseline: 1.0103x; 1.0103x over previous
"""Gemma4 MoE feed-forward on 8 Trainium2 NeuronCores.

Strategy: expert-parallel. E == n_cores == 8, so core e owns expert e's
weights (Wg[e], Wu[e], Wd[e]) and receives exactly the tokens routed to
expert e (gathered + transposed + padded on the host). Each core runs a
dense gated-FFN over its token batch:

    dT = Wd^T @ (gelu_tanh(Wg^T x^T) * (Wu^T x^T))        (all [*, C] layouts)

The host then scatter-adds routing_weight * dT^T back into the full
[T, H] output. Tokens that select the same expert in both slots are
deduplicated on the host (weights summed).

All matmul operands are bf16 (rel err ~4e-3 vs the 2e-2 gate): bf16
streams the moving operand ~20ns/MM faster than fp32r and halves HBM
traffic. The up phase runs i-outer / n-inner so each Wg/Wu tile is
DMA'd exactly once (n-outer re-streamed all up weights per token block
and saturated the 358 GB/s per-core HBM budget). DMAs are consolidated
into few large transfers: each dma_start costs ~650ns of serial
sequencer issue time, so the startup ramp is paced by issue count as
much as by bytes.
"""

import os
import sys

import numpy as np

for _p in ("/opt/trn_rl_repo", "/root/.axon_site/_ro/trn_rl_repo"):
    if os.path.isdir(_p) and _p not in sys.path:
        sys.path.append(_p)

T, H, I, E, K = 4096, 2048, 1024, 8, 2
NCORES = 8
KH = H // 128  # 16 k-tiles over the hidden dim
KI = I // 128  # 8 k-tiles over the intermediate dim

_PROGRAM_CACHE = {}
LAST_RESULT = None  # BassKernelResults of the most recent run (for test.py)
TRACE = False  # test.py sets this to capture an NTFF profile
TRACE_CORES = [0]

NWARM_A = int(os.environ.get("MOE_NWARM_A", "8"))
NWARM_B = int(os.environ.get("MOE_NWARM_B", "10"))
OTAIL = 128  # final down-group split width (shortens the last write chain)


def _tile_w_up(W):
    """[H, I] -> [KI, 128, KH*128] with [i, p, k*128+c] = W[k*128+p, i*128+c]:
    the i-tile's whole lhsT set is one DMA, 4KB contiguous per partition."""
    return np.ascontiguousarray(
        W.reshape(KH, 128, KI, 128).transpose(2, 1, 0, 3).reshape(KI, 128, KH * 128)
    )


def _tile_w_down(W):
    """[I, H] -> [KH, 128, KI*128], same scheme (contraction over I)."""
    return np.ascontiguousarray(
        W.reshape(KI, 128, KH, 128).transpose(2, 1, 0, 3).reshape(KH, 128, KI * 128)
    )


def _pick_config(max_count):
    """Minimal uniform token-block config: NT blocks of even width N with
    NT*N >= max_count, N <= 512 (PSUM bank limit) and N >= 256 (so the
    ~100ns LDWEIGHTS fully hides under the moving-operand stream)."""
    mc = max(max_count, 256)
    nt = -(-mc // 512)
    n = -(-mc // nt)
    n += n % 2
    return (nt * n, nt, n)  # (C, NT, N)


def _build_program(C, NT, N):
    import concourse.tile as tile
    from concourse import bacc, mybir
    from contextlib import ExitStack

    f32 = mybir.dt.float32
    bf16 = mybir.dt.bfloat16

    nc = bacc.Bacc("TRN2", target_bir_lowering=False, debug=False)

    # x arrives host-packed: [n, p, k*N+c] = x^T[k*128+p, n*N+c], so each
    # n-block is one contiguous KH*N*2 = ~16KB run per partition.
    xP = nc.dram_tensor("xP", [NT, 128, KH * N], bf16, kind="ExternalInput").ap()
    Wg_d = nc.dram_tensor("Wg", [KI, 128, KH * 128], bf16, kind="ExternalInput").ap()
    Wu_d = nc.dram_tensor("Wu", [KI, 128, KH * 128], bf16, kind="ExternalInput").ap()
    Wd_d = nc.dram_tensor("Wd", [KH, 128, KI * 128], bf16, kind="ExternalInput").ap()
    dT = nc.dram_tensor("dT", [H, C], f32, kind="ExternalOutput").ap()

    # Partition-major view: row a*128+p -> partition p, free index a.
    dT_p = dT.rearrange("(a p) c -> p a c", p=128)  # [128, KH, C]

    GELU = mybir.ActivationFunctionType.Gelu_apprx_tanh

    with tile.TileContext(nc) as tc, ExitStack() as ctx:
        xpool = ctx.enter_context(tc.tile_pool(name="x", bufs=1))
        wpool = ctx.enter_context(tc.tile_pool(name="w", bufs=3))
        apool = ctx.enter_context(tc.tile_pool(name="a", bufs=1))
        tpool = ctx.enter_context(tc.tile_pool(name="t", bufs=2))
        opool = ctx.enter_context(tc.tile_pool(name="o", bufs=4))
        wdpool = ctx.enter_context(tc.tile_pool(name="wd", bufs=4))

        w_tiles = {}

        def issue_w(i):
            wgt = wpool.tile([128, KH * 128], bf16, tag="wg", name=f"wg{i}")
            wut = wpool.tile([128, KH * 128], bf16, tag="wu", name=f"wu{i}")
            nc.sync.dma_start(wgt[:], Wg_d[i])
            nc.sync.dma_start(wut[:], Wu_d[i])
            w_tiles[i] = (wgt, wut)

        xts = {}

        def alloc_x(n):
            t = xpool.tile([128, KH * N], bf16, name=f"x{n}")
            xts[n] = t
            return t

        def issue_x_part(n, k0, k1):
            nc.sync.dma_start(
                xts[n][:, k0 * N : k1 * N], xP[n, :, k0 * N : k1 * N]
            )

        # Ramp emission order (sync ring is FIFO, drained in issue order):
        # the i=0 weights and n=0 x block are split into halves/quarters
        # interleaved in the order group (0,0) consumes them — the tile
        # framework tracks dependencies at slice granularity, so the real
        # stream starts as soon as the first ~1MB lands (~12.5us) instead
        # of waiting for the whole first-group working set. Then the
        # remaining x blocks, then the i=1,2 weight prefetches (x must
        # outrank them or the stream stalls at (i=0, n=1)).
        wg0 = wpool.tile([128, KH * 128], bf16, tag="wg", name="wg0")
        wu0 = wpool.tile([128, KH * 128], bf16, tag="wu", name="wu0")
        w_tiles[0] = (wg0, wu0)
        hw = (KH // 2) * 128
        for n in range(NT):
            alloc_x(n)
        nc.sync.dma_start(wg0[:, 0:hw], Wg_d[0, :, 0:hw])
        nc.sync.dma_start(wu0[:, 0:hw], Wu_d[0, :, 0:hw])
        issue_x_part(0, 0, KH // 4)
        issue_x_part(0, KH // 4, KH // 2)
        nc.sync.dma_start(wg0[:, hw:], Wg_d[0, :, hw:])
        nc.sync.dma_start(wu0[:, hw:], Wu_d[0, :, hw:])
        issue_x_part(0, KH // 2, 3 * KH // 4)
        issue_x_part(0, 3 * KH // 4, KH)
        for n in range(1, NT):
            for q in range(4):
                issue_x_part(n, q * KH // 4, (q + 1) * KH // 4)
        for i in (1, 2):
            if i < KI:
                issue_w(i)

        # PE clock-gate warmup: HAM starts at 1.2 GHz and un-throttles only
        # after ~3.4us of sustained activity, and real matmuls can't start
        # until the first weights + x land from HBM (~12-18us; the launch
        # timing jitters run-to-run by several us). Phase A: dummy bf16
        # matmuls on memset scratch need no DMA, run right at launch, and
        # flip HAM to 2.4 GHz. Phase B: dummy matmuls whose moving operand
        # is the (just-DMA'd) first half of wg0 — they begin exactly when
        # that DMA lands, so the bridge to the real stream self-times to
        # the DMA schedule instead of relying on a fixed count (a fixed
        # warmup either delays the stream or leaves a >3.4us PE gap that
        # re-throttles HAM, costing ~4us when the launch runs late).
        with (
            tc.tile_pool(name="warm", bufs=1) as wmpool,
            tc.tile_pool(name="warmps", bufs=1, space="PSUM") as wmpspool,
        ):
            wt = wmpool.tile([128, 512], bf16, name="warm_in")
            nc.vector.memset(wt[:], 0.0)
            wps = wmpspool.tile([128, 512], f32, name="warm_ps")
            for r in range(NWARM_A):
                nc.tensor.matmul(wps[:], wt[:, 0:128], wt[:], start=True, stop=True)
            for r in range(NWARM_B):
                s = (r % 2) * 512
                nc.tensor.matmul(
                    wps[:], wt[:, 0:128], wg0[:, s : s + 512], start=True, stop=True
                )

        aT = apool.tile([128, KI, C], bf16, name="aT")

        wd_tiles = {}

        def issue_wd(h):
            wdt = wdpool.tile([128, KI * 128], bf16, tag="wd", name=f"wd{h}")
            nc.sync.dma_start(wdt[:], Wd_d[h])
            wd_tiles[h] = wdt

        # Both PSUM pools stay open for the whole kernel (2*2 + 4 = 8
        # banks): closing gu before opening d would insert a drain barrier
        # (~1.5-3.5us of PE idle at the up->down transition).
        with (
            tc.tile_pool(name="gu", bufs=2, space="PSUM") as gupool,
            tc.tile_pool(name="d", bufs=4, space="PSUM") as dpool,
        ):
            # Up phase: i-outer so each weight tile is loaded exactly once.
            for i in range(KI):
                if i + 3 < KI and i + 3 not in w_tiles:
                    issue_w(i + 3)
                # prefetch the first down-phase weights near the end
                if i >= KI - 3 and (h := i - (KI - 3)) < 3:
                    issue_wd(h)
                if i not in w_tiles:
                    issue_w(i)
                wgt, wut = w_tiles.pop(i)
                for n in range(NT):
                    g_ps = gupool.tile([128, N], f32, tag="g", name=f"g{i}_{n}")
                    u_ps = gupool.tile([128, N], f32, tag="u", name=f"u{i}_{n}")
                    nsl = slice(n * N, (n + 1) * N)
                    for k in range(KH):
                        ksl = slice(k * 128, (k + 1) * 128)
                        xs = xts[n][:, k * N : (k + 1) * N]
                        nc.tensor.matmul(
                            g_ps[:], wgt[:, ksl], xs,
                            start=(k == 0), stop=(k == KH - 1),
                        )
                        nc.tensor.matmul(
                            u_ps[:], wut[:, ksl], xs,
                            start=(k == 0), stop=(k == KH - 1),
                        )
                    gel = tpool.tile([128, N], f32, tag="gel", name=f"gel{i}_{n}")
                    nc.scalar.activation(gel[:], g_ps[:], GELU)
                    nc.vector.tensor_mul(aT[:, i, nsl], gel[:], u_ps[:])

            # Down phase: d^T[h] = sum_ki Wd[ki,h]^T @ aT[ki]. Output DMAs
            # alternate between the two HWDGE rings (nc.scalar / nc.sync)
            # so HBM write receipts overlap and neither ring backs up. The
            # very last group is split into a wide piece and an OTAIL-wide
            # piece so the final copy+DMA+receipt chain is short.
            for h in range(KH):
                if h + 3 < KH and h + 3 not in wd_tiles:
                    issue_wd(h + 3)
                if h not in wd_tiles:
                    issue_wd(h)
                wdt = wd_tiles.pop(h)
                for n in range(NT):
                    last = h == KH - 1 and n == NT - 1
                    splits = (
                        [(0, N - OTAIL), (N - OTAIL, N)] if last else [(0, N)]
                    )
                    for si, (c0, c1) in enumerate(splits):
                        d_ps = dpool.tile(
                            [128, c1 - c0], f32, tag="d", name=f"d{h}_{n}_{si}"
                        )
                        for ki in range(KI):
                            nc.tensor.matmul(
                                d_ps[:],
                                wdt[:, ki * 128 : (ki + 1) * 128],
                                aT[:, ki, n * N + c0 : n * N + c1],
                                start=(ki == 0),
                                stop=(ki == KI - 1),
                            )
                        o = opool.tile(
                            [128, c1 - c0], f32, tag="o", name=f"o{h}_{n}_{si}"
                        )
                        nc.vector.tensor_copy(o[:], d_ps[:])
                        eng = nc.scalar if (h * NT + n + si) % 2 == 0 else nc.sync
                        eng.dma_start(
                            dT_p[:, h, n * N + c0 : n * N + c1], o[:]
                        )

    nc.compile()
    return nc


def _get_program(C, NT, N):
    key = (C, NT, N, NWARM_A, NWARM_B)
    if key not in _PROGRAM_CACHE:
        _PROGRAM_CACHE[key] = _build_program(C, NT, N)
    return _PROGRAM_CACHE[key]


def _ensure_ntff_hook():
    """Register the axon NTFF profile hook if the image's antenv lacks
    axon_hooks (see trn_agent_boot.trn_boot). Only needed when TRACE."""
    import types

    try:
        from antenv.axon_hooks import get_axon_ntff_profile_hook  # noqa: F401

        return
    except ImportError:
        pass
    import antenv
    from trn_agent_boot.trn_boot import _ntff_profile_via_ctypes

    hook = _ntff_profile_via_ctypes("/opt/axon/libaxon_pjrt.so")
    mod = types.ModuleType("antenv.axon_hooks")
    state = {"hook": hook}
    mod.set_axon_ntff_profile_hook = lambda h: state.__setitem__("hook", h)
    mod.get_axon_ntff_profile_hook = lambda: state["hook"]
    sys.modules["antenv.axon_hooks"] = mod
    antenv.axon_hooks = mod


def kernel(x, Wg, Wu, Wd, selected_experts, routing_weights):
    global LAST_RESULT
    import ml_dtypes
    from concourse.bass_utils import run_bass_kernel_spmd

    if TRACE:
        _ensure_ntff_hook()

    bf16 = ml_dtypes.bfloat16
    x = np.asarray(x, dtype=np.float32)
    Wg = np.asarray(Wg, dtype=np.float32)
    Wu = np.asarray(Wu, dtype=np.float32)
    Wd = np.asarray(Wd, dtype=np.float32)
    selected_experts = np.asarray(selected_experts)
    routing_weights = np.asarray(routing_weights, dtype=np.float32)

    # Host-side dispatch: per expert, the (deduplicated) token list and
    # summed routing weights.
    idx_list, w_list = [], []
    for e in range(E):
        m = selected_experts == e  # [T, K]
        idx = np.nonzero(m.any(axis=1))[0]
        w = (routing_weights * m).sum(axis=1)[idx]
        idx_list.append(idx)
        w_list.append(w.astype(np.float32))

    max_count = max(len(idx) for idx in idx_list)
    C, NT, N = _pick_config(max_count)

    nc = _get_program(C, NT, N)

    in_maps = []
    for e in range(E):
        idx = idx_list[e]
        xT = np.zeros((H, C), dtype=bf16)
        xT[:, : len(idx)] = x[idx].T.astype(bf16)
        # pack [n, p, k*N+c] = xT[k*128+p, n*N+c]
        xPk = (
            xT.reshape(KH, 128, NT, N).transpose(2, 1, 0, 3).reshape(NT, 128, KH * N)
        )
        in_maps.append(
            {
                "xP": np.ascontiguousarray(xPk),
                "Wg": _tile_w_up(Wg[e].astype(bf16)),
                "Wu": _tile_w_up(Wu[e].astype(bf16)),
                "Wd": _tile_w_down(Wd[e].astype(bf16)),
            }
        )

    res = run_bass_kernel_spmd(
        nc,
        in_maps,
        list(range(NCORES)),
        trace=TRACE,
        trace_cores=TRACE_CORES if TRACE else None,
    )
    LAST_RESULT = res

    out = np.zeros((T, H), dtype=np.float32)
    for e in range(E):
        idx = idx_list[e]
        dTe = res.results[e]["dT"]  # [H, C] fp32
        out[idx] += w_list[e][:, None] * dTe[:, : len(idx)].T
    return out
